# revision 1
# baseline (speedup 1.0000x reference)
"""MatAnyone memory-readout kernel for 8 Trainium2 NeuronCores.

Math (per batch b):
  sim[t,n]  = (-a_sq + two_ab - b_sq)[t,n] * ms[t] / sqrt(CK)
  aff       = softmax_t(sim)
  R[c,n]    = sum_t mv[c,t] * aff[t,n]
  out[c,n]  = R[c,n] * p[n] + lv[c,n] * (1 - p[n])

Sharding: 8 cores = 2 batches x 4 query-pixel shards (n = HW/4 = 576 each).
Single interleaved pass over 144 t-tiles; the two n-halves of 288 share each
t-tile's weights and one exp op. PSUM: sim pair 2 banks + 4 R accumulators +
2 Z accumulators = 8 banks exactly. Softmax runs with t on partitions:
  sim matmul:  lhsT = [mk^2 ; mk] (K=128=2*CK), rhs = [-qe/8 ; qe*qk/4]
  psum       -= b_sq/8 (DVE, broadcast tile)
  E           = Exp(psum * ms_t)      (ACT, per-partition scale)
  R, Z        = matmuls contracting t, accumulated across all 144 t-tiles
  out         = R * (p/Z) + lv * (1-p)
Softmax max-subtraction is skipped: sim <= 0 always (negative weighted L2
distance), and max_t sim ~ 0, so exp never overflows and Z >= exp(max) is
well-scaled.
"""

import sys

for _p in ("/opt/trn_rl_repo", "/root/.axon_site/_ro/trn_rl_repo"):
    if _p not in sys.path:
        sys.path.insert(0, _p)

from contextlib import ExitStack

import numpy as np
import ml_dtypes

import concourse.bass as bass
from concourse import mybir
from concourse.bacc import Bacc
from concourse.tile import TileContext
from concourse.bass_utils import run_bass_kernel_spmd

F32 = mybir.dt.float32
F32R = mybir.dt.float32r
BF16 = mybir.dt.bfloat16
FP16 = mybir.dt.float16
EXP = mybir.ActivationFunctionType.Exp

B, CK, CV, T, H, W = 2, 64, 256, 8, 48, 48
HW = H * W            # 2304
THW = T * HW          # 18432
NCORE = HW // 4       # 576 query pixels per core
NH = NCORE // 2       # 288 per n-half (psum-bank sized)
TT = THW // 128       # 144 t-tiles
MKCH = 4              # t-tiles per streamed M2 chunk
SKEW = 3              # software-pipeline skew (tiles) between exp and readout

_CACHE = {}


def _f32r(ap):
    return ap.bitcast(F32R)


def build_program():
    nc = Bacc(name="matanyone_knn")

    cz_h = nc.declare_dram_parameter("c_onesz", [128, 2], BF16, isOutput=False)
    cb_h = nc.declare_dram_parameter("c_onesb", [1, 128], F32R, isOutput=False)
    ce_h = nc.declare_dram_parameter("c_eighth", [CK, 128], F32R, isOutput=False)
    qk_h = nc.declare_dram_parameter("qk", [CK, NCORE], F32, isOutput=False)
    qe_h = nc.declare_dram_parameter("qe", [CK, NCORE], F32, isOutput=False)
    mk_h = nc.declare_dram_parameter("mk", [CK, THW], FP16, isOutput=False)
    ms_h = nc.declare_dram_parameter("msT", [128, TT], F32, isOutput=False)
    mv_h = nc.declare_dram_parameter("mvT", [THW, CV], BF16, isOutput=False)
    lv_h = nc.declare_dram_parameter("lv", [CV, NCORE], F32, isOutput=False)
    p_h = nc.declare_dram_parameter("p", [1, NCORE], F32, isOutput=False)
    out_h = nc.declare_dram_parameter("out", [CV, NCORE], F32, isOutput=True)

    with TileContext(nc) as tc, ExitStack() as ctx:
        persist = ctx.enter_context(tc.tile_pool(name="persist", bufs=1))
        mvpool = ctx.enter_context(tc.tile_pool(name="mv", bufs=1))
        m2pool = ctx.enter_context(tc.tile_pool(name="m2", bufs=2))
        epool = ctx.enter_context(tc.tile_pool(name="E", bufs=SKEW + 2))
        dpool = ctx.enter_context(tc.tile_pool(name="D", bufs=2))
        ps_sim = ctx.enter_context(tc.tile_pool(name="pssim", bufs=2, space="PSUM"))
        ps_acc = ctx.enter_context(tc.tile_pool(name="psacc", bufs=1, space="PSUM"))

        # ---- constants / setup -------------------------------------------
        ones_z = persist.tile([128, 2], BF16, tag="ones_z")      # Z matmul lhsT
        nc.sync.dma_start(out=ones_z[:], in_=cz_h[:])
        ones_b = persist.tile([1, 128], F32R, tag="ones_b")      # K=1 broadcast lhsT
        nc.sync.dma_start(out=ones_b[:], in_=cb_h[:])
        eighth = persist.tile([CK, 128], F32R, tag="eighth")     # b_sq/8 lhsT
        nc.sync.dma_start(out=eighth[:], in_=ce_h[:])

        ms_sb = persist.tile([128, TT], F32, tag="ms")
        nc.sync.dma_start(out=ms_sb[:], in_=ms_h[:])
        p_sb = persist.tile([1, NCORE], F32, tag="p")
        nc.sync.dma_start(out=p_sb[:], in_=p_h[:])

        q_sb = persist.tile([128, NCORE], FP16, tag="q")
        bsq_sb = persist.tile([128, NCORE], F32, tag="bsq")

        with tc.tile_pool(name="setup", bufs=1) as setup:
            qk_sb = setup.tile([CK, NCORE], F32, tag="qk")
            nc.sync.dma_start(out=qk_sb[:], in_=qk_h[:])
            qe_sb = setup.tile([CK, NCORE], F32, tag="qe")
            nc.sync.dma_start(out=qe_sb[:], in_=qe_h[:])
            t1 = setup.tile([CK, NCORE], F32, tag="t1")
            t2 = setup.tile([CK, NCORE], F32R, tag="t2")

            # copy-then-mul keeps each DVE op to a single cross-engine wait
            nc.vector.tensor_copy(t1[:], qk_sb[:])
            nc.vector.tensor_mul(t1[:], t1[:], qe_sb[:])               # qe*qk
            nc.vector.tensor_scalar_mul(q_sb[0:CK, :], qe_sb[:], -0.125)
            nc.vector.tensor_scalar_mul(q_sb[CK:128, :], t1[:], 0.25)
            nc.vector.tensor_mul(t2[:], t1[:], qk_sb[:])               # qe*qk^2

            for hh in (0, 1):
                pb = ps_sim.tile([128, NH], F32, tag="sim", name=f"pb{hh}")
                nc.tensor.matmul(pb[:], eighth[:], t2[:, hh * NH:(hh + 1) * NH],
                                 start=True, stop=True)
                nc.vector.tensor_copy(bsq_sb[:, hh * NH:(hh + 1) * NH], pb[:])

        fin = ctx.enter_context(tc.tile_pool(name="fin", bufs=1))
        lv0 = fin.tile([128, NCORE], F32, tag="lv0")
        nc.sync.dma_start(out=lv0[:], in_=lv_h[0:128, :])
        lv1 = fin.tile([128, NCORE], F32, tag="lv1")
        nc.sync.dma_start(out=lv1[:], in_=lv_h[128:256, :])

        # ---- resident mvT (chunks DMA'd inside the main loop) -----------
        mv_sb = mvpool.tile([128, TT * CV], BF16, tag="mvres")

        def load_mv_chunk(g):
            src = mv_h[g * 2048:(g + 1) * 2048, :].rearrange(
                "(j p) c -> p j c", p=128)
            dst = mv_sb[:, g * 16 * CV:(g + 1) * 16 * CV].rearrange(
                "p (j c) -> p j c", c=CV)
            nc.sync.dma_start(out=dst, in_=src)

        # ---- main interleaved pass -------------------------------------
        r_acc = {}
        for k in (0, 1):
            for hh in (0, 1):
                r_acc[k, hh] = ps_acc.tile([128, NH], F32, tag=f"r{k}{hh}",
                                           name=f"r{k}{hh}")
        z_acc = [ps_acc.tile([2, NH], F32, tag=f"z{hh}", name=f"z{hh}")
                 for hh in (0, 1)]

        e_tiles = {}
        m2c = None
        for t in range(TT + SKEW):
            if t < TT:
                if t % 16 == 0:
                    load_mv_chunk(t // 16)
                if t % MKCH == 0:
                    m2c = m2pool.tile([128, 128 * MKCH], FP16, tag="m2c")
                    nc.sync.dma_start(
                        out=m2c[CK:128, :],
                        in_=mk_h[:, t * 128:(t + MKCH) * 128])
                    nc.gpsimd.tensor_mul(m2c[0:CK, :], m2c[CK:128, :],
                                         m2c[CK:128, :])
                lw = m2c[:, (t % MKCH) * 128:(t % MKCH + 1) * 128]
                dt_ = dpool.tile([128, NCORE], F32, tag="D")
                for hh in (0, 1):
                    s = slice(hh * NH, (hh + 1) * NH)
                    sim = ps_sim.tile([128, NH], F32, tag="sim", name=f"sim{hh}")
                    nc.tensor.matmul(sim[:], lw, q_sb[:, s],
                                     start=True, stop=True)
                    nc.vector.tensor_sub(dt_[:, s], sim[:], bsq_sb[:, s])
                e = epool.tile([128, NCORE], BF16, tag="E")
                nc.scalar.activation(e[:], dt_[:], EXP, scale=ms_sb[:, t:t + 1])
                e_tiles[t] = e
            if t >= SKEW:
                tc_ = t - SKEW
                e = e_tiles.pop(tc_)
                st, sp = (tc_ == 0), (tc_ == TT - 1)
                for k in (0, 1):
                    lwk = mv_sb[:, tc_ * CV + k * 128:tc_ * CV + (k + 1) * 128]
                    for hh in (0, 1):
                        nc.tensor.matmul(r_acc[k, hh][:], lwk,
                                         e[:, hh * NH:(hh + 1) * NH],
                                         start=st, stop=sp)
                for hh in (0, 1):
                    nc.tensor.matmul(z_acc[hh][:], ones_z[:],
                                     e[:, hh * NH:(hh + 1) * NH],
                                     start=st, stop=sp)

        # ---- finalize ----------------------------------------------------
        rz = fin.tile([1, NCORE], F32, tag="rz")
        nc.vector.reciprocal(rz[:, 0:NH], z_acc[0][0:1, :])
        nc.vector.reciprocal(rz[:, NH:2 * NH], z_acc[1][0:1, :])
        w1 = fin.tile([1, NCORE], F32R, tag="w1")
        nc.vector.tensor_mul(w1[:], rz[:], p_sb[:])            # p / Z
        w2 = fin.tile([1, NCORE], F32R, tag="w2")
        nc.vector.tensor_scalar_mul(w2[:], p_sb[:], -1.0)
        nc.vector.tensor_scalar_add(w2[:], w2[:], 1.0)         # 1 - p

        w1s = fin.tile([128, NCORE], F32, tag="w1s")
        w2s = fin.tile([128, NCORE], F32, tag="w2s")
        for w, ws in ((w1, w1s), (w2, w2s)):
            for hh in (0, 1):
                s = slice(hh * NH, (hh + 1) * NH)
                wps = ps_sim.tile([128, NH], F32, tag="sim", name=f"wps{hh}")
                nc.tensor.matmul(wps[:], ones_b[:], w[:, s],
                                 start=True, stop=True)
                nc.vector.tensor_copy(ws[:, s], wps[:])

        for k, lvt in ((0, lv0), (1, lv1)):
            o = fin.tile([128, NCORE], F32, tag="O", bufs=2)
            tmp = fin.tile([128, NCORE], F32, tag="tmp", bufs=2)
            for hh in (0, 1):
                s = slice(hh * NH, (hh + 1) * NH)
                nc.vector.tensor_mul(o[:, s], r_acc[k, hh][:], w1s[:, s])
            nc.vector.tensor_mul(tmp[:], lvt[:], w2s[:])
            nc.vector.tensor_add(o[:], o[:], tmp[:])
            nc.sync.dma_start(out=out_h[k * 128:(k + 1) * 128, :], in_=o[:])

    nc.finalize()
    return nc


def _get_program():
    if "nc" not in _CACHE:
        _CACHE["nc"] = build_program()
    return _CACHE["nc"]


def _make_in_maps(query_key, query_selection, memory_key, memory_shrinkage,
                  msk_value, uncert_prob):
    qk = np.asarray(query_key, np.float32).reshape(B, CK, HW)
    qe = np.asarray(query_selection, np.float32).reshape(B, CK, HW)
    mk = np.asarray(memory_key, np.float32).reshape(B, CK, THW)
    ms = np.asarray(memory_shrinkage, np.float32).reshape(B, THW)
    mv = np.asarray(msk_value, np.float32).reshape(B, CV, THW)
    lv = np.asarray(msk_value, np.float32).reshape(B, CV, T, HW)[:, :, T - 1, :]
    p = np.asarray(uncert_prob, np.float32).reshape(B, HW)

    in_maps = []
    for core in range(8):
        b, s = divmod(core, 4)
        sl = slice(s * NCORE, (s + 1) * NCORE)
        in_maps.append({
            "c_onesz": np.ones((128, 2), ml_dtypes.bfloat16),
            "c_onesb": np.ones((1, 128), np.float32),
            "c_eighth": np.full((CK, 128), 0.125, np.float32),
            "qk": np.ascontiguousarray(qk[b, :, sl]),
            "qe": np.ascontiguousarray(qe[b, :, sl]),
            "mk": np.ascontiguousarray(mk[b]).astype(np.float16),
            "msT": np.ascontiguousarray(ms[b].reshape(TT, 128).T),
            "mvT": np.ascontiguousarray(mv[b].T).astype(ml_dtypes.bfloat16),
            "lv": np.ascontiguousarray(lv[b, :, sl]),
            "p": np.ascontiguousarray(p[b, sl]).reshape(1, NCORE),
        })
    return in_maps


def kernel(**inputs):
    nc = _get_program()
    in_maps = _make_in_maps(**inputs)
    res = run_bass_kernel_spmd(nc, in_maps, list(range(8)))
    out = np.empty((B, 1, CV, HW), np.float32)
    for core in range(8):
        b, s = divmod(core, 4)
        out[b, 0, :, s * NCORE:(s + 1) * NCORE] = res.results[core]["out"]
    return out.reshape(B, 1, CV, H, W)


if __name__ == "__main__":
    rng = np.random.default_rng(0)
    dummy = {
        "query_key": rng.standard_normal((B, CK, H, W), np.float32),
        "query_selection": rng.random((B, CK, H, W), np.float32),
        "memory_key": rng.standard_normal((B, CK, T, H, W), np.float32),
        "memory_shrinkage": rng.random((B, 1, T, H, W), np.float32),
        "msk_value": rng.standard_normal((B, 1, CV, T, H, W), np.float32),
        "uncert_prob": rng.random((B, 1, H, W), np.float32),
    }
    out = kernel(**dummy)
    print("out", out.shape, out.dtype, float(np.abs(out).mean()))



# revision 10
# speedup vs baseline: 1.4751x; 1.4751x over previous
"""MatAnyone memory-readout kernel for 8 Trainium2 NeuronCores (fp8 DoubleRow).

Math (per batch b, query pixel n, memory slot t):
  sim[t,n] = ms[t]*(-a_sq + 2ab - b_sq)[t,n]/sqrt(CK)
  aff      = softmax_t(sim);  R[c,n] = sum_t mv[c,t]*aff[t,n]
  out[c,n] = R[c,n]*p[n] + lv[c,n]*(1-p[n])

Sharding: 8 cores = 2 batches x 4 query-pixel shards (n = 576 per core).

Per-core plan (fp8 e4m3 DoubleRow matmuls, K=2x contraction per pass):
  sim: lhsT = [ms*mk^2/2 | 2*ms*mk] pairs + 65th row (2ms | ms/4) folding
       -b_sq*ms/8 (with fp8 residual slot), rhs = [-qe/2 | qe*qk/4] pairs +
       (-b_sq/8 | 8*residual). One DR matmul per (t-tile, n-half) ->
       psum = 2*sim. ACT: E = exp(0.5*psum) -> fp8, one instr per t-tile.
  R:   t-tiles paired (K=256 over t): lhsT = mv pair-chunk [128,2,128],
       rhs = E-pair [128,2,288]. 4 DR matmuls per pair, accumulated over
       all 72 pairs in psum.
  Z:   even pairs on PE (DR ones-weight, M=1) into psum rows; odd pairs on
       DVE (G2 += E-pair); merged at the end by bf16 ones-matmuls that
       accumulate partition-sums of G2 into the same psum Z region.
  PSUM (8 banks, matmuls write across bank boundaries):
       RZ tile [128,1728] = R quarters @0/288/576/864 + Z @1152/1440 (4 banks)
       sim pool [128,576] x 2 bufs (2 banks each).
  out = R*(p/Z) + lv*(1-p); lv*(1-p) precomputed on host.
"""

import sys

for _p in ("/opt/trn_rl_repo", "/root/.axon_site/_ro/trn_rl_repo"):
    if _p not in sys.path:
        sys.path.insert(0, _p)

from contextlib import ExitStack

import numpy as np
import ml_dtypes

import concourse.bass as bass
from concourse import mybir
from concourse.bacc import Bacc
from concourse.tile import TileContext
from concourse.bass_utils import run_bass_kernel_spmd

F32 = mybir.dt.float32
BF16 = mybir.dt.bfloat16
FP8 = mybir.dt.float8e4
EXP = mybir.ActivationFunctionType.Exp
DR = mybir.MatmulPerfMode.DoubleRow
E4M3 = ml_dtypes.float8_e4m3

B, CK, CV, T, H, W = 2, 64, 256, 8, 48, 48
HW = H * W            # 2304
THW = T * HW          # 18432
NCORE = HW // 4       # 576 query pixels per core
NH = NCORE // 2       # 288 per n-half
TT = THW // 128       # 144 t-tiles
NPAIR = TT // 2       # 72 t-tile pairs
SKEW = 2              # pairs of lag between exp and readout
CHP = 8               # pairs per streamed mkw chunk
Z_PE_MOD = 10**9          # pair a accumulates Z on PE iff a % Z_PE_MOD == 0

_CACHE = {}


def _fp8(x):
    return np.clip(x, -240.0, 240.0).astype(E4M3)


def build_program():
    nc = Bacc(name="matanyone_fp8dr")

    qw_h = nc.declare_dram_parameter("qw", [65, 2 * NCORE], FP8, isOutput=False)
    mkw_h = nc.declare_dram_parameter("mkw", [65, TT * 256], FP8, isOutput=False)
    mvw_h = nc.declare_dram_parameter("mvw", [128, NPAIR * 512], FP8,
                                      isOutput=False)
    lvw2_h = nc.declare_dram_parameter("lvw2", [CV, NCORE], F32, isOutput=False)
    p_h = nc.declare_dram_parameter("p", [1, NCORE], F32, isOutput=False)
    cz_h = nc.declare_dram_parameter("c_onesz", [128, 32], FP8, isOutput=False)
    cb_h = nc.declare_dram_parameter("c_onesb", [128, 1], BF16, isOutput=False)
    cb1_h = nc.declare_dram_parameter("c_onesb1", [1, 128], BF16, isOutput=False)
    out_h = nc.declare_dram_parameter("out", [CV, NCORE], F32, isOutput=True)
    zdbg_h = nc.declare_dram_parameter("zdbg", [1, NCORE], F32, isOutput=True)
    sdbg_h = nc.declare_dram_parameter("sdbg", [128, NCORE], F32, isOutput=True)
    edbg_h = nc.declare_dram_parameter("edbg", [128, 2 * NCORE], F32, isOutput=True)
    rdbg_h = nc.declare_dram_parameter("rdbg", [128, NCORE], F32, isOutput=True)

    with TileContext(nc) as tc, ExitStack() as ctx:
        persist = ctx.enter_context(tc.tile_pool(name="persist", bufs=1))
        mvpool = ctx.enter_context(tc.tile_pool(name="mv", bufs=1))
        m2pool = ctx.enter_context(tc.tile_pool(name="m2", bufs=2))
        epool = ctx.enter_context(tc.tile_pool(name="E", bufs=SKEW + 2))
        ps_rz = ctx.enter_context(tc.tile_pool(name="psrz", bufs=1, space="PSUM"))
        ps_sim = ctx.enter_context(tc.tile_pool(name="pssim", bufs=2,
                                                space="PSUM"))

        # ---- persistent inputs / constants --------------------------------
        qw = persist.tile([65, 2 * NCORE], FP8, tag="qw")
        nc.sync.dma_start(out=qw[:], in_=qw_h[:])
        onesz = persist.tile([128, 32], FP8, tag="onesz")
        nc.sync.dma_start(out=onesz[:], in_=cz_h[:])
        onesb = persist.tile([128, 1], BF16, tag="onesb")
        nc.sync.dma_start(out=onesb[:], in_=cb_h[:])
        onesb1 = persist.tile([1, 128], BF16, tag="onesb1")
        nc.sync.dma_start(out=onesb1[:], in_=cb1_h[:])
        p_sb = persist.tile([1, NCORE], F32, tag="p")
        nc.sync.dma_start(out=p_sb[:], in_=p_h[:])
        lvw2 = []
        for k in (0, 1):
            t = persist.tile([128, NCORE], F32, tag=f"lvw2{k}")
            nc.sync.dma_start(out=t[:], in_=lvw2_h[k * 128:(k + 1) * 128, :])
            lvw2.append(t)
        g2 = persist.tile([128, 2 * NCORE], F32, tag="g2")

        qw3 = qw.rearrange("p (i n) -> p i n", i=2)
        onesz3 = onesz.rearrange("p (i m) -> p i m", i=2)

        # resident mv weights, DMA'd in chunks inside the loop
        mvw = mvpool.tile([128, NPAIR * 512], FP8, tag="mvw")

        # ---- psum layout ---------------------------------------------------
        # RZ: R quarters (k,hh) @ (2k+hh)*512, each within one bank
        rz = ps_rz.tile([128, 2048], F32, tag="rz")

        e_tiles = {}
        mkc = None

        def pair_front(a):
            nonlocal mkc
            if a % CHP == 0:
                g = a // CHP
                mkc = m2pool.tile([65, CHP * 512], FP8, tag="mkc")
                nc.sync.dma_start(out=mkc[:],
                                  in_=mkw_h[:, g * CHP * 512:(g + 1) * CHP * 512])
                src = mvw_h[:, g * CHP * 512:(g + 1) * CHP * 512]
                nc.sync.dma_start(out=mvw[:, g * CHP * 512:(g + 1) * CHP * 512],
                                  in_=src)
            e = epool.tile([128, 2 * NCORE], FP8, tag="E")
            for j in (0, 1):
                tau = 2 * a + j
                wsl = mkc[:, (a % CHP) * 512 + j * 256:(a % CHP) * 512
                          + (j + 1) * 256]
                w3 = wsl.rearrange("p (i m) -> p i m", i=2)
                sim = ps_sim.tile([128, 1024], F32, tag="sim")
                for hh in (0, 1):
                    nc.tensor.matmul(sim[:, hh * 512:hh * 512 + NH], w3,
                                     qw3[:, :, hh * NH:(hh + 1) * NH],
                                     start=True, stop=True, perf_mode=DR)
                sim3 = sim.rearrange("p (i n) -> p i n", i=2)[:, :, 0:NH]
                e2 = e[:, j * NCORE:(j + 1) * NCORE].rearrange(
                    "p (i n) -> p i n", i=2)
                nc.scalar.activation(e2, sim3, EXP, scale=0.5)
                if a == 0 and j == 0:
                    sdbg = persist.tile([128, NCORE], F32, tag="sdbg")
                    nc.vector.tensor_copy(
                        sdbg.rearrange("p (i n) -> p i n", i=2), sim3)
                    nc.sync.dma_start(out=sdbg_h[:], in_=sdbg[:])
            if a == 0:
                edbg = persist.tile([128, 2 * NCORE], F32, tag="edbg")
                nc.vector.tensor_copy(edbg[:], e[:])
                nc.sync.dma_start(out=edbg_h[:], in_=edbg[:])
            e_tiles[a] = e

        def pair_back(a):
            e = e_tiles.pop(a)
            e3 = e.rearrange("p (i n) -> p i n", i=2)
            st, sp = (a == 0), (a == NPAIR - 1)
            for k in (0, 1):
                wsl = mvw[:, a * 512 + k * 256:a * 512 + (k + 1) * 256]
                w3 = wsl.rearrange("p (i m) -> p i m", i=2)
                for hh in (0, 1):
                    q = (2 * k + hh) * 512
                    nc.tensor.matmul(
                        rz[:, q:q + NH],
                        w3, e3[:, :, hh * NH:(hh + 1) * NH],
                        start=st, stop=sp, perf_mode=DR)
            if a == 0:
                nc.vector.tensor_copy(g2[:], e[:])
            else:
                nc.vector.tensor_add(g2[:], g2[:], e[:])

        for a in range(NPAIR + SKEW):
            if a < NPAIR:
                pair_front(a)
            if a >= SKEW:
                pair_back(a - SKEW)

        # ---- finalize ------------------------------------------------------
        fin = ctx.enter_context(tc.tile_pool(name="fin", bufs=1))
        # fold G2 partition-sums into psum: Z = ones^T @ G2 (both i-halves)
        gb = fin.tile([128, 2 * NCORE], BF16, tag="gb")
        nc.vector.tensor_copy(gb[:], g2[:])
        zt = ps_sim.tile([128, 1024], F32, tag="sim")
        for hh in (0, 1):
            for i in (0, 1):
                nc.tensor.matmul(
                    zt[0:1, hh * 512:hh * 512 + NH], onesb[:],
                    gb[:, i * NCORE + hh * NH:i * NCORE + (hh + 1) * NH],
                    start=(i == 0), stop=(i == 1))

        zrow = fin.tile([1, NCORE], F32, tag="zrow")
        zt3 = zt.rearrange("p (i n) -> p i n", i=2)[0:1, :, 0:NH]
        nc.vector.tensor_copy(zrow.rearrange("p (i n) -> p i n", i=2), zt3)
        nc.sync.dma_start(out=zdbg_h[:], in_=zrow[:])
        rdbg = fin.tile([128, NCORE], F32, tag="rdbg")
        nc.vector.tensor_copy(
            rdbg.rearrange("p (i n) -> p i n", i=2),
            rz[:, 0:1024].rearrange("p (i n) -> p i n", i=2)[:, :, 0:NH])
        nc.sync.dma_start(out=rdbg_h[:], in_=rdbg[:])
        rzv = fin.tile([1, NCORE], F32, tag="rzv")
        nc.vector.reciprocal(rzv[:], zrow[:])
        w1 = fin.tile([1, NCORE], BF16, tag="w1")
        nc.vector.tensor_mul(w1[:], rzv[:], p_sb[:])          # p / Z

        w1s = fin.tile([128, NCORE], F32, tag="w1s")
        wt = ps_sim.tile([128, 1024], F32, tag="sim")
        for hh in (0, 1):
            nc.tensor.matmul(wt[:, hh * 512:hh * 512 + NH], onesb1[:],
                             w1[:, hh * NH:(hh + 1) * NH],
                             start=True, stop=True)
        wt3 = wt.rearrange("p (i n) -> p i n", i=2)[:, :, 0:NH]
        nc.vector.tensor_copy(w1s.rearrange("p (i n) -> p i n", i=2), wt3)

        for k in (0, 1):
            o = fin.tile([128, NCORE], F32, tag="o", bufs=2)
            rk = rz[:, k * 1024:k * 1024 + 1024].rearrange(
                "p (i n) -> p i n", i=2)[:, :, 0:NH]
            nc.vector.tensor_mul(o.rearrange("p (i n) -> p i n", i=2), rk,
                                 w1s.rearrange("p (i n) -> p i n", i=2))
            nc.vector.tensor_add(o[:], o[:], lvw2[k][:])
            nc.sync.dma_start(out=out_h[k * 128:(k + 1) * 128, :], in_=o[:])

    nc.finalize()
    return nc


def _get_program():
    if "nc" not in _CACHE:
        _CACHE["nc"] = build_program()
    return _CACHE["nc"]


def _make_in_maps(query_key, query_selection, memory_key, memory_shrinkage,
                  msk_value, uncert_prob):
    qk = np.asarray(query_key, np.float32).reshape(B, CK, HW)
    qe = np.asarray(query_selection, np.float32).reshape(B, CK, HW)
    mk = np.asarray(memory_key, np.float32).reshape(B, CK, THW)
    ms = np.asarray(memory_shrinkage, np.float32).reshape(B, THW)
    mv = np.asarray(msk_value, np.float32).reshape(B, CV, THW)
    lv = np.asarray(msk_value, np.float32).reshape(B, CV, T, HW)[:, :, T - 1, :]
    p = np.asarray(uncert_prob, np.float32).reshape(B, HW)

    # per-batch sim weights: [65, TT, 2, 128]
    mkw_b = []
    mvw_b = []
    for b in range(B):
        mk3 = mk[b].reshape(CK, TT, 128)               # [c, tau, m]
        ms3 = ms[b].reshape(TT, 128)                   # [tau, m]
        mkw = np.empty((65, TT, 2, 128), np.float32)
        mkw[:CK, :, 0, :] = ms3[None] * mk3 * mk3 * 0.5
        mkw[:CK, :, 1, :] = 2.0 * ms3[None] * mk3
        mkw[64, :, 0, :] = 2.0 * ms3
        mkw[64, :, 1, :] = 0.25 * ms3
        mkw_b.append(_fp8(mkw).reshape(65, TT * 256))
        # mv DR weights: [p, a, k, i, m] = mv[k*128+m, (2a+i)*128+p]
        tmp = mv[b].reshape(2, 128, NPAIR, 2, 128)     # [k, m, a, i, p]
        mvw = tmp.transpose(4, 2, 0, 3, 1).reshape(128, NPAIR * 512)
        mvw_b.append(_fp8(mvw))

    in_maps = []
    for core in range(8):
        b, s = divmod(core, 4)
        sl = slice(s * NCORE, (s + 1) * NCORE)
        qks, qes = qk[b, :, sl], qe[b, :, sl]
        bsq = np.einsum("cn,cn->n", qes, qks * qks)    # [576]
        qwf = np.empty((65, 2, NCORE), np.float32)
        qwf[:CK, 0] = -0.5 * qes
        qwf[:CK, 1] = 0.25 * qes * qks
        b0 = _fp8(-bsq / 8.0)
        qwf[64, 0] = b0.astype(np.float32)
        qwf[64, 1] = 8.0 * (-bsq / 8.0 - b0.astype(np.float32))
        ps = p[b, sl]
        in_maps.append({
            "qw": _fp8(qwf).reshape(65, 2 * NCORE),
            "mkw": mkw_b[b],
            "mvw": mvw_b[b],
            "lvw2": np.ascontiguousarray(lv[b, :, sl] * (1.0 - ps)[None, :]),
            "p": np.ascontiguousarray(ps).reshape(1, NCORE),
            "c_onesz": np.ones((128, 32), E4M3),
            "c_onesb": np.ones((128, 1), ml_dtypes.bfloat16),
            "c_onesb1": np.ones((1, 128), ml_dtypes.bfloat16),
        })
    return in_maps


def kernel(**inputs):
    nc = _get_program()
    in_maps = _make_in_maps(**inputs)
    res = run_bass_kernel_spmd(nc, in_maps, list(range(8)))
    out = np.empty((B, 1, CV, HW), np.float32)
    for core in range(8):
        b, s = divmod(core, 4)
        out[b, 0, :, s * NCORE:(s + 1) * NCORE] = res.results[core]["out"]
    return out.reshape(B, 1, CV, H, W)


if __name__ == "__main__":
    rng = np.random.default_rng(0)
    dummy = {
        "query_key": rng.standard_normal((B, CK, H, W)).astype(np.float32),
        "query_selection": rng.random((B, CK, H, W)).astype(np.float32),
        "memory_key": rng.standard_normal((B, CK, T, H, W)).astype(np.float32),
        "memory_shrinkage": rng.random((B, 1, T, H, W)).astype(np.float32),
        "msk_value": rng.standard_normal((B, 1, CV, T, H, W)).astype(np.float32),
        "uncert_prob": rng.random((B, 1, H, W)).astype(np.float32),
    }
    out = kernel(**dummy)
    print("out", out.shape, out.dtype, float(np.abs(out).mean()))


# revision 11
# speedup vs baseline: 1.5029x; 1.0188x over previous
"""MatAnyone memory-readout kernel for 8 Trainium2 NeuronCores (fp8 DoubleRow).

Math (per batch b, query pixel n, memory slot t):
  sim[t,n] = ms[t]*(-a_sq + 2ab - b_sq)[t,n]/sqrt(CK)
  aff      = softmax_t(sim);  R[c,n] = sum_t mv[c,t]*aff[t,n]
  out[c,n] = R[c,n]*p[n] + lv[c,n]*(1-p[n])

Sharding: 8 cores = 2 batches x 4 query-pixel shards (n = 576 per core).

Per-core plan (fp8 e4m3 DoubleRow matmuls, K=2x contraction per pass):
  sim: lhsT = [ms*mk^2/2 | 2*ms*mk] pairs + 65th row (2ms | ms/4) folding
       -b_sq*ms/8 (with fp8 residual slot), rhs = [-qe/2 | qe*qk/4] pairs +
       (-b_sq/8 | 8*residual). One DR matmul per (t-tile, n-half) ->
       psum = 2*sim. ACT: E = exp(0.5*psum) -> fp8, one instr per t-tile.
  R:   t-tiles paired (K=256 over t): lhsT = mv pair-chunk [128,2,128],
       rhs = E-pair [128,2,288]. 4 DR matmuls per pair, accumulated over
       all 72 pairs in psum.
  Z:   even pairs on PE (DR ones-weight, M=1) into psum rows; odd pairs on
       DVE (G2 += E-pair); merged at the end by bf16 ones-matmuls that
       accumulate partition-sums of G2 into the same psum Z region.
  PSUM (8 banks, matmuls write across bank boundaries):
       RZ tile [128,1728] = R quarters @0/288/576/864 + Z @1152/1440 (4 banks)
       sim pool [128,576] x 2 bufs (2 banks each).
  out = R*(p/Z) + lv*(1-p); lv*(1-p) precomputed on host.
"""

import sys

for _p in ("/opt/trn_rl_repo", "/root/.axon_site/_ro/trn_rl_repo"):
    if _p not in sys.path:
        sys.path.insert(0, _p)

from contextlib import ExitStack

import numpy as np
import ml_dtypes

import concourse.bass as bass
from concourse import mybir
from concourse.bacc import Bacc
from concourse.tile import TileContext
from concourse.bass_utils import run_bass_kernel_spmd

F32 = mybir.dt.float32
BF16 = mybir.dt.bfloat16
FP8 = mybir.dt.float8e4
EXP = mybir.ActivationFunctionType.Exp
DR = mybir.MatmulPerfMode.DoubleRow
E4M3 = ml_dtypes.float8_e4m3

B, CK, CV, T, H, W = 2, 64, 256, 8, 48, 48
HW = H * W            # 2304
THW = T * HW          # 18432
NCORE = HW // 4       # 576 query pixels per core
NH = NCORE // 2       # 288 per n-half
TT = THW // 128       # 144 t-tiles
NPAIR = TT // 2       # 72 t-tile pairs
SKEW = 2              # pairs of lag between exp and readout
CHP = 8               # pairs per streamed mkw chunk
Z_PE_MOD = 10**9          # pair a accumulates Z on PE iff a % Z_PE_MOD == 0

_CACHE = {}


def _fp8(x):
    return np.clip(x, -240.0, 240.0).astype(E4M3)


def build_program():
    nc = Bacc(name="matanyone_fp8dr")

    qw_h = nc.declare_dram_parameter("qw", [128, 2 * NCORE], FP8, isOutput=False)
    mkw_h = nc.declare_dram_parameter("mkw", [128, TT * 256], FP8,
                                      isOutput=False)
    mvw_h = nc.declare_dram_parameter("mvw", [128, NPAIR * 512], FP8,
                                      isOutput=False)
    lvw2_h = nc.declare_dram_parameter("lvw2", [CV, NCORE], F32, isOutput=False)
    p_h = nc.declare_dram_parameter("p", [1, NCORE], F32, isOutput=False)
    cz_h = nc.declare_dram_parameter("c_onesz", [128, 32], FP8, isOutput=False)
    cb_h = nc.declare_dram_parameter("c_onesb", [128, 1], BF16, isOutput=False)
    cb1_h = nc.declare_dram_parameter("c_onesb1", [1, 128], BF16, isOutput=False)
    out_h = nc.declare_dram_parameter("out", [CV, NCORE], F32, isOutput=True)


    with TileContext(nc) as tc, ExitStack() as ctx:
        persist = ctx.enter_context(tc.tile_pool(name="persist", bufs=1))
        mvpool = ctx.enter_context(tc.tile_pool(name="mv", bufs=1))
        m2pool = ctx.enter_context(tc.tile_pool(name="m2", bufs=2))
        epool = ctx.enter_context(tc.tile_pool(name="E", bufs=SKEW + 2))
        ps_rz = ctx.enter_context(tc.tile_pool(name="psrz", bufs=1, space="PSUM"))
        ps_sim = ctx.enter_context(tc.tile_pool(name="pssim", bufs=2,
                                                space="PSUM"))

        # ---- persistent inputs / constants --------------------------------
        qw = persist.tile([128, 2 * NCORE], FP8, tag="qw")
        nc.sync.dma_start(out=qw[:], in_=qw_h[:])
        onesz = persist.tile([128, 32], FP8, tag="onesz")
        nc.sync.dma_start(out=onesz[:], in_=cz_h[:])
        onesb = persist.tile([128, 1], BF16, tag="onesb")
        nc.sync.dma_start(out=onesb[:], in_=cb_h[:])
        onesb1 = persist.tile([1, 128], BF16, tag="onesb1")
        nc.sync.dma_start(out=onesb1[:], in_=cb1_h[:])
        p_sb = persist.tile([1, NCORE], F32, tag="p")
        nc.sync.dma_start(out=p_sb[:], in_=p_h[:])
        lvw2 = []
        for k in (0, 1):
            t = persist.tile([128, NCORE], F32, tag=f"lvw2{k}")
            nc.sync.dma_start(out=t[:], in_=lvw2_h[k * 128:(k + 1) * 128, :])
            lvw2.append(t)
        g2 = persist.tile([128, 2 * NCORE], F32, tag="g2")

        qw3 = qw.rearrange("p (i n) -> p i n", i=2)
        onesz3 = onesz.rearrange("p (i m) -> p i m", i=2)

        # resident mv weights, DMA'd in chunks inside the loop
        mvw = mvpool.tile([128, NPAIR * 512], FP8, tag="mvw")

        # ---- psum layout ---------------------------------------------------
        # RZ: R quarters (k,hh) @ (2k+hh)*512, each within one bank
        rz = ps_rz.tile([128, 2048], F32, tag="rz")

        e_tiles = {}
        mkc = None

        def pair_front(a):
            nonlocal mkc
            if a % CHP == 0:
                g = a // CHP
                mkc = m2pool.tile([128, CHP * 512], FP8, tag="mkc")
                nc.sync.dma_start(out=mkc[:],
                                  in_=mkw_h[:, g * CHP * 512:(g + 1) * CHP * 512])
                src = mvw_h[:, g * CHP * 512:(g + 1) * CHP * 512]
                nc.sync.dma_start(out=mvw[:, g * CHP * 512:(g + 1) * CHP * 512],
                                  in_=src)
            e = epool.tile([128, 2 * NCORE], FP8, tag="E")
            for j in (0, 1):
                tau = 2 * a + j
                wsl = mkc[:, (a % CHP) * 512 + j * 256:(a % CHP) * 512
                          + (j + 1) * 256]
                w3 = wsl.rearrange("p (i m) -> p i m", i=2)
                sim = ps_sim.tile([128, 1024], F32, tag="sim")
                for hh in (0, 1):
                    nc.tensor.matmul(sim[:, hh * 512:hh * 512 + NH], w3,
                                     qw3[:, :, hh * NH:(hh + 1) * NH],
                                     start=True, stop=True, perf_mode=DR)
                sim3 = sim.rearrange("p (i n) -> p i n", i=2)[:, :, 0:NH]
                e2 = e[:, j * NCORE:(j + 1) * NCORE].rearrange(
                    "p (i n) -> p i n", i=2)
                nc.scalar.activation(e2, sim3, EXP, scale=0.5)
            e_tiles[a] = e

        def pair_back(a):
            e = e_tiles.pop(a)
            e3 = e.rearrange("p (i n) -> p i n", i=2)
            st, sp = (a == 0), (a == NPAIR - 1)
            for k in (0, 1):
                wsl = mvw[:, a * 512 + k * 256:a * 512 + (k + 1) * 256]
                w3 = wsl.rearrange("p (i m) -> p i m", i=2)
                for hh in (0, 1):
                    q = (2 * k + hh) * 512
                    nc.tensor.matmul(
                        rz[:, q:q + NH],
                        w3, e3[:, :, hh * NH:(hh + 1) * NH],
                        start=st, stop=sp, perf_mode=DR)
            if a % Z_PE_MOD == 0:
                for s in range(4):
                    nc.tensor.matmul(
                        rz[0:1, s * 512 + 288:s * 512 + 432],
                        onesz3[:, :, 0:1], e3[:, :, s * 144:(s + 1) * 144],
                        start=(a == 0), stop=False, perf_mode=DR)
            elif a == 1:
                nc.vector.tensor_copy(g2[:], e[:])
            else:
                nc.vector.tensor_add(g2[:], g2[:], e[:])

        for a in range(NPAIR + SKEW):
            if a < NPAIR:
                pair_front(a)
            if a >= SKEW:
                pair_back(a - SKEW)

        # ---- finalize ------------------------------------------------------
        fin = ctx.enter_context(tc.tile_pool(name="fin", bufs=1))
        # fold G2 partition-sums into psum: Z = ones^T @ G2 (both i-halves)
        gb = fin.tile([128, 2 * NCORE], BF16, tag="gb")
        nc.vector.tensor_copy(gb[:], g2[:])
        for s in range(4):
            for i in (0, 1):
                nc.tensor.matmul(
                    rz[0:1, s * 512 + 288:s * 512 + 432], onesb[:],
                    gb[:, i * NCORE + s * 144:i * NCORE + (s + 1) * 144],
                    start=False, stop=(i == 1))

        zrow = fin.tile([1, NCORE], F32, tag="zrow")
        rz4 = rz.rearrange("p (s c) -> p s c", c=512)[0:1, :, 288:432]
        nc.vector.tensor_copy(zrow.rearrange("p (s n) -> p s n", s=4), rz4)
        rzv = fin.tile([1, NCORE], F32, tag="rzv")
        nc.vector.reciprocal(rzv[:], zrow[:])
        w1 = fin.tile([1, NCORE], BF16, tag="w1")
        nc.vector.tensor_mul(w1[:], rzv[:], p_sb[:])          # p / Z

        w1s = fin.tile([128, NCORE], F32, tag="w1s")
        wt = ps_sim.tile([128, 1024], F32, tag="sim")
        for hh in (0, 1):
            nc.tensor.matmul(wt[:, hh * 512:hh * 512 + NH], onesb1[:],
                             w1[:, hh * NH:(hh + 1) * NH],
                             start=True, stop=True)
        wt3 = wt.rearrange("p (i n) -> p i n", i=2)[:, :, 0:NH]
        nc.vector.tensor_copy(w1s.rearrange("p (i n) -> p i n", i=2), wt3)

        for k in (0, 1):
            o = fin.tile([128, NCORE], F32, tag="o", bufs=2)
            rk = rz[:, k * 1024:k * 1024 + 1024].rearrange(
                "p (i n) -> p i n", i=2)[:, :, 0:NH]
            nc.vector.tensor_mul(o.rearrange("p (i n) -> p i n", i=2), rk,
                                 w1s.rearrange("p (i n) -> p i n", i=2))
            nc.vector.tensor_add(o[:], o[:], lvw2[k][:])
            nc.sync.dma_start(out=out_h[k * 128:(k + 1) * 128, :], in_=o[:])

    nc.finalize()
    return nc


def _get_program():
    if "nc" not in _CACHE:
        _CACHE["nc"] = build_program()
    return _CACHE["nc"]


def _make_in_maps(query_key, query_selection, memory_key, memory_shrinkage,
                  msk_value, uncert_prob):
    qk = np.asarray(query_key, np.float32).reshape(B, CK, HW)
    qe = np.asarray(query_selection, np.float32).reshape(B, CK, HW)
    mk = np.asarray(memory_key, np.float32).reshape(B, CK, THW)
    ms = np.asarray(memory_shrinkage, np.float32).reshape(B, THW)
    mv = np.asarray(msk_value, np.float32).reshape(B, CV, THW)
    lv = np.asarray(msk_value, np.float32).reshape(B, CV, T, HW)[:, :, T - 1, :]
    p = np.asarray(uncert_prob, np.float32).reshape(B, HW)

    # per-batch sim weights: [65, TT, 2, 128]
    mkw_b = []
    mvw_b = []
    for b in range(B):
        mk3 = mk[b].reshape(CK, TT, 128)               # [c, tau, m]
        ms3 = ms[b].reshape(TT, 128)                   # [tau, m]
        mkw = np.zeros((128, TT, 2, 128), np.float32)
        mkw[:CK, :, 0, :] = ms3[None] * mk3 * mk3 * 0.5
        mkw[:CK, :, 1, :] = 2.0 * ms3[None] * mk3
        mkw[64, :, 0, :] = 2.0 * ms3
        mkw[64, :, 1, :] = 0.25 * ms3
        mkw_b.append(_fp8(mkw).reshape(128, TT * 256))
        # mv DR weights: [p, a, k, i, m] = mv[k*128+m, (2a+i)*128+p]
        tmp = mv[b].reshape(2, 128, NPAIR, 2, 128)     # [k, m, a, i, p]
        mvw = tmp.transpose(4, 2, 0, 3, 1).reshape(128, NPAIR * 512)
        mvw_b.append(_fp8(mvw))

    in_maps = []
    for core in range(8):
        b, s = divmod(core, 4)
        sl = slice(s * NCORE, (s + 1) * NCORE)
        qks, qes = qk[b, :, sl], qe[b, :, sl]
        bsq = np.einsum("cn,cn->n", qes, qks * qks)    # [576]
        qwf = np.zeros((128, 2, NCORE), np.float32)
        qwf[:CK, 0] = -0.5 * qes
        qwf[:CK, 1] = 0.25 * qes * qks
        b0 = _fp8(-bsq / 8.0)
        qwf[64, 0] = b0.astype(np.float32)
        qwf[64, 1] = 8.0 * (-bsq / 8.0 - b0.astype(np.float32))
        ps = p[b, sl]
        in_maps.append({
            "qw": _fp8(qwf).reshape(128, 2 * NCORE),
            "mkw": mkw_b[b],
            "mvw": mvw_b[b],
            "lvw2": np.ascontiguousarray(lv[b, :, sl] * (1.0 - ps)[None, :]),
            "p": np.ascontiguousarray(ps).reshape(1, NCORE),
            "c_onesz": np.ones((128, 32), E4M3),
            "c_onesb": np.ones((128, 1), ml_dtypes.bfloat16),
            "c_onesb1": np.ones((1, 128), ml_dtypes.bfloat16),
        })
    return in_maps


def kernel(**inputs):
    nc = _get_program()
    in_maps = _make_in_maps(**inputs)
    res = run_bass_kernel_spmd(nc, in_maps, list(range(8)))
    out = np.empty((B, 1, CV, HW), np.float32)
    for core in range(8):
        b, s = divmod(core, 4)
        out[b, 0, :, s * NCORE:(s + 1) * NCORE] = res.results[core]["out"]
    return out.reshape(B, 1, CV, H, W)


if __name__ == "__main__":
    rng = np.random.default_rng(0)
    dummy = {
        "query_key": rng.standard_normal((B, CK, H, W)).astype(np.float32),
        "query_selection": rng.random((B, CK, H, W)).astype(np.float32),
        "memory_key": rng.standard_normal((B, CK, T, H, W)).astype(np.float32),
        "memory_shrinkage": rng.random((B, 1, T, H, W)).astype(np.float32),
        "msk_value": rng.standard_normal((B, 1, CV, T, H, W)).astype(np.float32),
        "uncert_prob": rng.random((B, 1, H, W)).astype(np.float32),
    }
    out = kernel(**dummy)
    print("out", out.shape, out.dtype, float(np.abs(out).mean()))


# revision 13
# speedup vs baseline: 1.6301x; 1.0847x over previous
"""MatAnyone memory-readout kernel for 8 Trainium2 NeuronCores (fp8 DoubleRow).

Math (per batch b, query pixel n, memory slot t):
  sim[t,n] = ms[t]*(-a_sq + 2ab - b_sq)[t,n]/sqrt(CK)
  aff      = softmax_t(sim);  R[c,n] = sum_t mv[c,t]*aff[t,n]
  out[c,n] = R[c,n]*p[n] + lv[c,n]*(1-p[n])

Sharding: 8 cores = 2 batches x 4 query-pixel shards (n = 576 per core).

Per-core plan (fp8 e4m3 DoubleRow matmuls, K=2x contraction per pass):
  sim: lhsT = [ms*mk^2/2 | 2*ms*mk] pairs + 65th row (2ms | ms/4) folding
       -b_sq*ms/8 (with fp8 residual slot), rhs = [-qe/2 | qe*qk/4] pairs +
       (-b_sq/8 | 8*residual). One DR matmul per (t-tile, n-half) ->
       psum = 2*sim. ACT: E = exp(0.5*psum) -> fp8, one instr per t-tile.
  R:   t-tiles paired (K=256 over t): lhsT = mv pair-chunk [128,2,128],
       rhs = E-pair [128,2,288]. 4 DR matmuls per pair, accumulated over
       all 72 pairs in psum.
  Z:   even pairs on PE (DR ones-weight, M=1) into psum rows; odd pairs on
       DVE (G2 += E-pair); merged at the end by bf16 ones-matmuls that
       accumulate partition-sums of G2 into the same psum Z region.
  PSUM (8 banks, matmuls write across bank boundaries):
       RZ tile [128,1728] = R quarters @0/288/576/864 + Z @1152/1440 (4 banks)
       sim pool [128,576] x 2 bufs (2 banks each).
  out = R*(p/Z) + lv*(1-p); lv*(1-p) precomputed on host.
"""

import sys

for _p in ("/opt/trn_rl_repo", "/root/.axon_site/_ro/trn_rl_repo"):
    if _p not in sys.path:
        sys.path.insert(0, _p)

from contextlib import ExitStack

import numpy as np
import ml_dtypes

import concourse.bass as bass
from concourse import mybir
from concourse.bacc import Bacc
from concourse.tile import TileContext
from concourse.bass_utils import run_bass_kernel_spmd

F32 = mybir.dt.float32
BF16 = mybir.dt.bfloat16
FP8 = mybir.dt.float8e4
EXP = mybir.ActivationFunctionType.Exp
DR = mybir.MatmulPerfMode.DoubleRow
E4M3 = ml_dtypes.float8_e4m3

B, CK, CV, T, H, W = 2, 64, 256, 8, 48, 48
HW = H * W            # 2304
THW = T * HW          # 18432
NCORE = HW // 4       # 576 query pixels per core
NH = NCORE // 2       # 288 per n-half
TT = THW // 128       # 144 t-tiles
NPAIR = TT // 2       # 72 t-tile pairs
SKEW = 2              # pairs of lag between exp and readout
CHP = 8               # pairs per streamed mkw chunk
Z_PE_MOD = 10**9          # pair a accumulates Z on PE iff a % Z_PE_MOD == 0

_CACHE = {}


def _fp8(x):
    return np.clip(x, -240.0, 240.0).astype(E4M3)


def build_program():
    nc = Bacc(name="matanyone_fp8dr")

    qw_h = nc.declare_dram_parameter("qw", [128, 2 * NCORE], FP8, isOutput=False)
    mkw_h = nc.declare_dram_parameter("mkw", [128, TT * 256], FP8,
                                      isOutput=False)
    mvw_h = nc.declare_dram_parameter("mvw", [128, NPAIR * 512], FP8,
                                      isOutput=False)
    lvw2_h = nc.declare_dram_parameter("lvw2", [CV, NCORE], F32, isOutput=False)
    p_h = nc.declare_dram_parameter("p", [1, NCORE], F32, isOutput=False)
    cz_h = nc.declare_dram_parameter("c_onesz", [128, 32], FP8, isOutput=False)
    cb_h = nc.declare_dram_parameter("c_onesb", [128, 1], BF16, isOutput=False)
    cb1_h = nc.declare_dram_parameter("c_onesb1", [1, 128], BF16, isOutput=False)
    out_h = nc.declare_dram_parameter("out", [CV, NCORE], F32, isOutput=True)


    with TileContext(nc) as tc, ExitStack() as ctx:
        persist = ctx.enter_context(tc.tile_pool(name="persist", bufs=1))
        ps_rz0 = ctx.enter_context(tc.tile_pool(name="psrz", bufs=1,
                                                space="PSUM"))
        mvpool = ctx.enter_context(tc.tile_pool(name="mv", bufs=1))
        m2pool = ctx.enter_context(tc.tile_pool(name="m2", bufs=2))
        epool = ctx.enter_context(tc.tile_pool(name="E", bufs=SKEW + 2))
        ps_sim = ctx.enter_context(tc.tile_pool(name="pssim", bufs=2,
                                                space="PSUM"))

        # RZ: R quarters (k,hh) @ (2k+hh)*512 + Z segs @ s*512+288
        rz = ps_rz0.tile([128, 2048], F32, tag="rz")

        # ---- persistent inputs / constants --------------------------------
        qw = persist.tile([128, 2 * NCORE], FP8, tag="qw")
        nc.sync.dma_start(out=qw[:], in_=qw_h[:])
        onesz = persist.tile([128, 32], FP8, tag="onesz")
        nc.sync.dma_start(out=onesz[:], in_=cz_h[:])
        onesb = persist.tile([128, 1], BF16, tag="onesb")
        nc.sync.dma_start(out=onesb[:], in_=cb_h[:])
        onesb1 = persist.tile([1, 128], BF16, tag="onesb1")
        nc.sync.dma_start(out=onesb1[:], in_=cb1_h[:])
        g2 = persist.tile([128, 2 * NCORE], F32, tag="g2")

        qw3 = qw.rearrange("p (i n) -> p i n", i=2)
        onesz3 = onesz.rearrange("p (i m) -> p i m", i=2)

        # PE warmup: tiny matmuls into the Z-gap region keep HAM at 8/8
        # through the initial DMA wait; pair 0's Z matmul resets the region.
        for _w in range(60):
            nc.tensor.matmul(rz[0:1, 288:304], onesz3[:, :, 0:1], onesz3[:],
                             start=True, stop=True, perf_mode=DR)

        # resident mv weights, DMA'd in chunks inside the loop
        mvw = mvpool.tile([128, NPAIR * 512], FP8, tag="mvw")

        e_tiles = {}
        mkc = None

        def pair_front(a):
            nonlocal mkc
            if a % CHP == 0:
                g = a // CHP
                mkc = m2pool.tile([128, CHP * 512], FP8, tag="mkc")
                if g == 0:
                    for q in range(CHP):
                        nc.sync.dma_start(
                            out=mkc[:, q * 512:(q + 1) * 512],
                            in_=mkw_h[:, q * 512:(q + 1) * 512])
                        nc.sync.dma_start(
                            out=mvw[:, q * 512:(q + 1) * 512],
                            in_=mvw_h[:, q * 512:(q + 1) * 512])
                else:
                    nc.sync.dma_start(
                        out=mkc[:],
                        in_=mkw_h[:, g * CHP * 512:(g + 1) * CHP * 512])
                    nc.sync.dma_start(
                        out=mvw[:, g * CHP * 512:(g + 1) * CHP * 512],
                        in_=mvw_h[:, g * CHP * 512:(g + 1) * CHP * 512])
            e = epool.tile([128, 2 * NCORE], FP8, tag="E")
            for j in (0, 1):
                tau = 2 * a + j
                wsl = mkc[:, (a % CHP) * 512 + j * 256:(a % CHP) * 512
                          + (j + 1) * 256]
                w3 = wsl.rearrange("p (i m) -> p i m", i=2)
                sim = ps_sim.tile([128, 1024], F32, tag="sim")
                for hh in (0, 1):
                    nc.tensor.matmul(sim[:, hh * 512:hh * 512 + NH], w3,
                                     qw3[:, :, hh * NH:(hh + 1) * NH],
                                     start=True, stop=True, perf_mode=DR)
                sim3 = sim.rearrange("p (i n) -> p i n", i=2)[:, :, 0:NH]
                e2 = e[:, j * NCORE:(j + 1) * NCORE].rearrange(
                    "p (i n) -> p i n", i=2)
                nc.scalar.activation(e2, sim3, EXP, scale=0.5)
            e_tiles[a] = e

        def pair_back(a):
            e = e_tiles.pop(a)
            e3 = e.rearrange("p (i n) -> p i n", i=2)
            st, sp = (a == 0), (a == NPAIR - 1)
            for k in (0, 1):
                wsl = mvw[:, a * 512 + k * 256:a * 512 + (k + 1) * 256]
                w3 = wsl.rearrange("p (i m) -> p i m", i=2)
                for hh in (0, 1):
                    q = (2 * k + hh) * 512
                    nc.tensor.matmul(
                        rz[:, q:q + NH],
                        w3, e3[:, :, hh * NH:(hh + 1) * NH],
                        start=st, stop=sp, perf_mode=DR)
            if a % Z_PE_MOD == 0:
                for s in range(4):
                    nc.tensor.matmul(
                        rz[0:1, s * 512 + 288:s * 512 + 432],
                        onesz3[:, :, 0:1], e3[:, :, s * 144:(s + 1) * 144],
                        start=(a == 0), stop=False, perf_mode=DR)
            elif a == 1:
                nc.vector.tensor_copy(g2[:], e[:])
            else:
                nc.vector.tensor_add(g2[:], g2[:], e[:])

        for a in range(NPAIR + SKEW):
            if a < NPAIR:
                pair_front(a)
            if a >= SKEW:
                pair_back(a - SKEW)

        # ---- finalize ------------------------------------------------------
        fin = ctx.enter_context(tc.tile_pool(name="fin", bufs=1))
        p_sb = persist.tile([1, NCORE], F32, tag="p")
        nc.sync.dma_start(out=p_sb[:], in_=p_h[:])
        lvw2 = []
        for k in (0, 1):
            t = persist.tile([128, NCORE], F32, tag=f"lvw2{k}")
            nc.sync.dma_start(out=t[:], in_=lvw2_h[k * 128:(k + 1) * 128, :])
            lvw2.append(t)
        # fold G2 partition-sums into psum: Z = ones^T @ G2 (both i-halves)
        gb = fin.tile([128, 2 * NCORE], BF16, tag="gb")
        nc.vector.tensor_copy(gb[:], g2[:])
        for s in range(4):
            for i in (0, 1):
                nc.tensor.matmul(
                    rz[0:1, s * 512 + 288:s * 512 + 432], onesb[:],
                    gb[:, i * NCORE + s * 144:i * NCORE + (s + 1) * 144],
                    start=False, stop=(i == 1))

        # 1/(2Z) on the scalar engine straight from psum (p has 2x folded in)
        rzv = fin.tile([1, NCORE], F32, tag="rzv")
        rz4 = rz.rearrange("p (s c) -> p s c", c=512)[0:1, :, 288:432]
        eng = nc.scalar
        eng.add_instruction(mybir.InstActivation(
            name=nc.get_next_instruction_name(),
            func=mybir.ActivationFunctionType.Reciprocal,
            ins=[eng.lower_ap(rz4),
                 mybir.ImmediateValue(dtype=mybir.dt.float32, value=0.0),
                 mybir.ImmediateValue(dtype=mybir.dt.float32, value=2.0),
                 mybir.ImmediateValue(dtype=mybir.dt.float32, value=0.0)],
            outs=[eng.lower_ap(rzv.rearrange("p (s n) -> p s n", s=4))]))
        w1 = fin.tile([1, NCORE], BF16, tag="w1")
        nc.vector.tensor_mul(w1[:], rzv[:], p_sb[:])          # 2p / 2Z

        w1s = fin.tile([128, NCORE], F32, tag="w1s")
        wt = ps_sim.tile([128, 1024], F32, tag="sim")
        for hh in (0, 1):
            nc.tensor.matmul(wt[:, hh * 512:hh * 512 + NH], onesb1[:],
                             w1[:, hh * NH:(hh + 1) * NH],
                             start=True, stop=True)
        wt3 = wt.rearrange("p (i n) -> p i n", i=2)[:, :, 0:NH]
        nc.vector.tensor_copy(w1s.rearrange("p (i n) -> p i n", i=2), wt3)

        for k in (0, 1):
            o = fin.tile([128, NCORE], F32, tag="o", bufs=2)
            rk = rz[:, k * 1024:k * 1024 + 1024].rearrange(
                "p (i n) -> p i n", i=2)[:, :, 0:NH]
            nc.vector.tensor_mul(o.rearrange("p (i n) -> p i n", i=2), rk,
                                 w1s.rearrange("p (i n) -> p i n", i=2))
            nc.vector.tensor_add(o[:], o[:], lvw2[k][:])
            nc.sync.dma_start(out=out_h[k * 128:(k + 1) * 128, :], in_=o[:])

    nc.finalize()
    return nc


def _get_program():
    if "nc" not in _CACHE:
        _CACHE["nc"] = build_program()
    return _CACHE["nc"]


def _make_in_maps(query_key, query_selection, memory_key, memory_shrinkage,
                  msk_value, uncert_prob):
    qk = np.asarray(query_key, np.float32).reshape(B, CK, HW)
    qe = np.asarray(query_selection, np.float32).reshape(B, CK, HW)
    mk = np.asarray(memory_key, np.float32).reshape(B, CK, THW)
    ms = np.asarray(memory_shrinkage, np.float32).reshape(B, THW)
    mv = np.asarray(msk_value, np.float32).reshape(B, CV, THW)
    lv = np.asarray(msk_value, np.float32).reshape(B, CV, T, HW)[:, :, T - 1, :]
    p = np.asarray(uncert_prob, np.float32).reshape(B, HW)

    # per-batch sim weights: [65, TT, 2, 128]
    mkw_b = []
    mvw_b = []
    for b in range(B):
        mk3 = mk[b].reshape(CK, TT, 128)               # [c, tau, m]
        ms3 = ms[b].reshape(TT, 128)                   # [tau, m]
        mkw = np.zeros((128, TT, 2, 128), np.float32)
        mkw[:CK, :, 0, :] = ms3[None] * mk3 * mk3 * 0.5
        mkw[:CK, :, 1, :] = 2.0 * ms3[None] * mk3
        mkw[64, :, 0, :] = 2.0 * ms3
        mkw[64, :, 1, :] = 0.25 * ms3
        mkw_b.append(_fp8(mkw).reshape(128, TT * 256))
        # mv DR weights: [p, a, k, i, m] = mv[k*128+m, (2a+i)*128+p]
        tmp = mv[b].reshape(2, 128, NPAIR, 2, 128)     # [k, m, a, i, p]
        mvw = tmp.transpose(4, 2, 0, 3, 1).reshape(128, NPAIR * 512)
        mvw_b.append(_fp8(mvw))

    in_maps = []
    for core in range(8):
        b, s = divmod(core, 4)
        sl = slice(s * NCORE, (s + 1) * NCORE)
        qks, qes = qk[b, :, sl], qe[b, :, sl]
        bsq = np.einsum("cn,cn->n", qes, qks * qks)    # [576]
        qwf = np.zeros((128, 2, NCORE), np.float32)
        qwf[:CK, 0] = -0.5 * qes
        qwf[:CK, 1] = 0.25 * qes * qks
        b0 = _fp8(-bsq / 8.0)
        qwf[64, 0] = b0.astype(np.float32)
        qwf[64, 1] = 8.0 * (-bsq / 8.0 - b0.astype(np.float32))
        ps = p[b, sl]
        in_maps.append({
            "qw": _fp8(qwf).reshape(128, 2 * NCORE),
            "mkw": mkw_b[b],
            "mvw": mvw_b[b],
            "lvw2": np.ascontiguousarray(lv[b, :, sl] * (1.0 - ps)[None, :]),
            "p": np.ascontiguousarray(ps).reshape(1, NCORE),
            "c_onesz": np.ones((128, 32), E4M3),
            "c_onesb": np.ones((128, 1), ml_dtypes.bfloat16),
            "c_onesb1": np.ones((1, 128), ml_dtypes.bfloat16),
        })
    return in_maps


def kernel(**inputs):
    nc = _get_program()
    in_maps = _make_in_maps(**inputs)
    res = run_bass_kernel_spmd(nc, in_maps, list(range(8)))
    out = np.empty((B, 1, CV, HW), np.float32)
    for core in range(8):
        b, s = divmod(core, 4)
        out[b, 0, :, s * NCORE:(s + 1) * NCORE] = res.results[core]["out"]
    return out.reshape(B, 1, CV, H, W)


if __name__ == "__main__":
    rng = np.random.default_rng(0)
    dummy = {
        "query_key": rng.standard_normal((B, CK, H, W)).astype(np.float32),
        "query_selection": rng.random((B, CK, H, W)).astype(np.float32),
        "memory_key": rng.standard_normal((B, CK, T, H, W)).astype(np.float32),
        "memory_shrinkage": rng.random((B, 1, T, H, W)).astype(np.float32),
        "msk_value": rng.standard_normal((B, 1, CV, T, H, W)).astype(np.float32),
        "uncert_prob": rng.random((B, 1, H, W)).astype(np.float32),
    }
    out = kernel(**dummy)
    print("out", out.shape, out.dtype, float(np.abs(out).mean()))
